# revision 3
# baseline (speedup 1.0000x reference)
"""Trainium2 Bass kernel for BaseModelWithEmbedding (3-branch LSTM + dense).

Model (per batch row b):
    hour_e = time_emb[hour_idx]            # [T, H]
    week_e = week_emb[week_idx]            # [T, H]
    h_sp   = LSTM(spatial; W_sp, U_sp, b_sp)  last hidden  [H]
    h_h    = LSTM(hour_e;  W_h,  U_h,  b_h)   last hidden  [H]
    h_w    = LSTM(week_e;  W_w,  U_w,  b_w)   last hidden  [H]
    out[b] = concat(h_sp, h_h, h_w) @ fc_W + fc_b

Sharding: pure data parallel, batch 256 -> 8 cores x 32.

Device layout (per core, batch-major):
  - The three LSTM "chains" are stacked on partition slots 0-31 / 32-63 /
    64-95 so elementwise gate math runs as single [96, .] ops.
  - Gate columns are host-permuted from (i,f,g,o) to (i,f,o,g) so one
    Sigmoid covers cols 0:384 and one Tanh covers 384:512.
  - xz (input contribution incl. bias) is computed by PE matmuls with a
    small stationary operand per step: spatial uses [x_t; 1] (K=3) against
    [W_sp; b_sp]; the embedding LSTMs use one-hot codes (K=24 / K=7)
    against precomputed tables (emb @ W + b), so the xz add is free PSUM
    accumulation and no [B,T,H] embedding tensor is ever materialized.
  - The three chains' matmuls are col-tiled (tile_position) so they run
    concurrently on the 128x128 PE array.
  - Recurrent matmul: z[32c:32c+32] += hT[:, 32c:32c+32].T @ U_c.
  - h is transposed back each step with one PE transpose ([96,128] ->
    [128,96]) + one PSUM->SBUF copy to feed the next step's stationary.
"""

import os
import sys

import numpy as np

for _p in ("/opt/trn_rl_repo",):
    if _p not in sys.path and os.path.isdir(_p):
        sys.path.insert(0, _p)

B, T, H = 256, 512, 128
NCORES = 8
BC = B // NCORES  # 32
H4 = 4 * H  # 512
WIN = 64  # timesteps per DMA window

_CACHE: dict = {}


def _gate_perm():
    """Column permutation (i,f,g,o) -> (i,f,o,g) on a 4H axis."""
    i = np.arange(H)
    return np.concatenate([i, H + i, 3 * H + i, 2 * H + i])


def _build_program(t_steps: int):
    import concourse.bacc as bacc
    import concourse.mybir as mybir
    from concourse.masks import make_identity
    from concourse.tile import TileContext

    FP = mybir.dt.float32
    Sig = mybir.ActivationFunctionType.Sigmoid
    Tah = mybir.ActivationFunctionType.Tanh

    nc = bacc.Bacc("TRN2", target_bir_lowering=False, debug=False)

    # DRAM tensors
    d_u_sp = nc.dram_tensor("u_sp", [H, H4], FP, kind="ExternalInput")
    d_u_h = nc.dram_tensor("u_h", [H, H4], FP, kind="ExternalInput")
    d_u_w = nc.dram_tensor("u_w", [H, H4], FP, kind="ExternalInput")
    d_waug = nc.dram_tensor("waug", [3, H4], FP, kind="ExternalInput")
    d_txzh = nc.dram_tensor("txzh", [24, H4], FP, kind="ExternalInput")
    d_txzw = nc.dram_tensor("txzw", [7, H4], FP, kind="ExternalInput")
    d_xaug = nc.dram_tensor("xaug", [t_steps, 3, BC], FP, kind="ExternalInput")
    d_ohh = nc.dram_tensor("ohh", [t_steps, 24, BC], FP, kind="ExternalInput")
    d_ohw = nc.dram_tensor("ohw", [t_steps, 7, BC], FP, kind="ExternalInput")
    d_fcw = nc.dram_tensor("fcw", [96, H], FP, kind="ExternalInput")
    d_fcb = nc.dram_tensor("fcb", [BC, 1], FP, kind="ExternalInput")
    d_out = nc.dram_tensor("out", [BC, 1], FP, kind="ExternalOutput")

    n_win = (t_steps + WIN - 1) // WIN

    with TileContext(nc) as tc:
        with (
            tc.tile_pool(name="consts", bufs=1) as consts,
            tc.tile_pool(name="state", bufs=1) as state,
            tc.tile_pool(name="gates", bufs=2) as gates,
            tc.tile_pool(name="win", bufs=2) as win,
            tc.tile_pool(name="zps", bufs=4, space="PSUM") as zps,
            tc.tile_pool(name="hps", bufs=2, space="PSUM") as hps,
        ):
            u_sp = consts.tile([H, H4], FP)
            u_h = consts.tile([H, H4], FP)
            u_w = consts.tile([H, H4], FP)
            waug = consts.tile([3, H4], FP)
            txzh = consts.tile([24, H4], FP)
            txzw = consts.tile([7, H4], FP)
            fcw = consts.tile([96, H], FP)
            fcb = consts.tile([BC, 1], FP)
            ident = consts.tile([96, 96], FP)

            nc.sync.dma_start(u_sp[:], d_u_sp.ap())
            nc.sync.dma_start(u_h[:], d_u_h.ap())
            nc.sync.dma_start(u_w[:], d_u_w.ap())
            nc.sync.dma_start(waug[:], d_waug.ap())
            nc.sync.dma_start(txzh[:], d_txzh.ap())
            nc.sync.dma_start(txzw[:], d_txzw.ap())
            nc.sync.dma_start(fcw[:], d_fcw.ap())
            nc.sync.dma_start(fcb[:], d_fcb.ap())
            make_identity(nc, ident[:])

            # Persistent state: hT [H, 96] (chain c at cols 32c:32c+32), c [96, H]
            hT = state.tile([H, 96], FP)
            cst = state.tile([96, H], FP)
            nc.vector.memset(hT[:], 0.0)
            nc.vector.memset(cst[:], 0.0)

            h_cur = None
            for w in range(n_win):
                t0 = w * WIN
                t1 = min(t_steps, t0 + WIN)
                nt = t1 - t0
                xw = win.tile([3, WIN * BC], FP, tag="xw")
                ohhw = win.tile([24, WIN * BC], FP, tag="ohhw")
                ohww = win.tile([7, WIN * BC], FP, tag="ohww")
                nc.sync.dma_start(
                    xw[:, : nt * BC].rearrange("k (t b) -> k t b", b=BC),
                    d_xaug.ap()[t0:t1].rearrange("t k b -> k t b"),
                )
                nc.sync.dma_start(
                    ohhw[:, : nt * BC].rearrange("k (t b) -> k t b", b=BC),
                    d_ohh.ap()[t0:t1].rearrange("t k b -> k t b"),
                )
                nc.sync.dma_start(
                    ohww[:, : nt * BC].rearrange("k (t b) -> k t b", b=BC),
                    d_ohw.ap()[t0:t1].rearrange("t k b -> k t b"),
                )

                for tt in range(nt):
                    sl = slice(tt * BC, (tt + 1) * BC)
                    z = zps.tile([96, H4], FP, tag="z")
                    # xz: start accumulation group per 32-partition slice
                    nc.tensor.matmul(
                        z[0:32], xw[:, sl], waug[:], start=True, stop=False,
                        tile_position=(0, 0),
                    )
                    nc.tensor.matmul(
                        z[32:64], ohhw[:, sl], txzh[:], start=True, stop=False,
                        tile_position=(0, 32),
                    )
                    nc.tensor.matmul(
                        z[64:96], ohww[:, sl], txzw[:], start=True, stop=False,
                        tile_position=(0, 64),
                    )
                    # recurrent part: z += h @ U
                    nc.tensor.matmul(
                        z[0:32], hT[:, 0:32], u_sp[:], start=False, stop=True,
                        tile_position=(0, 0),
                    )
                    nc.tensor.matmul(
                        z[32:64], hT[:, 32:64], u_h[:], start=False, stop=True,
                        tile_position=(0, 32),
                    )
                    nc.tensor.matmul(
                        z[64:96], hT[:, 64:96], u_w[:], start=False, stop=True,
                        tile_position=(0, 64),
                    )
                    # gates: cols 0:128 i, 128:256 f, 256:384 o, 384:512 g
                    sg = gates.tile([96, H4], FP, tag="sg")
                    nc.scalar.activation(sg[:, 0 : 3 * H], z[:, 0 : 3 * H], Sig)
                    nc.scalar.activation(sg[:, 3 * H : H4], z[:, 3 * H : H4], Tah)
                    # c = f*c + i*g~
                    t0m = gates.tile([96, H], FP, tag="t0m")
                    t1m = gates.tile([96, H], FP, tag="t1m")
                    nc.vector.tensor_mul(t0m[:], cst[:], sg[:, H : 2 * H])
                    nc.vector.tensor_mul(t1m[:], sg[:, 0:H], sg[:, 3 * H : H4])
                    nc.vector.tensor_add(cst[:], t0m[:], t1m[:])
                    # h = o * tanh(c)
                    tct = gates.tile([96, H], FP, tag="tct")
                    nc.scalar.activation(tct[:], cst[:], Tah)
                    hh = gates.tile([96, H], FP, tag="hh")
                    nc.vector.tensor_mul(hh[:], sg[:, 2 * H : 3 * H], tct[:])
                    h_cur = hh
                    # transpose h back for next step's stationary
                    hTp = hps.tile([H, 96], FP, tag="hTp")
                    nc.tensor.transpose(hTp[:], hh[:], ident[:])
                    nc.vector.tensor_copy(hT[:], hTp[:])

            # tail: out[b] = sum_c h[c*32+b, :] . fcw[c*32+b, :] + fc_b
            prod = state.tile([96, H], FP)
            dot = state.tile([96, 1], FP)
            al = state.tile([BC, 4], FP)
            res = state.tile([BC, 1], FP)
            nc.vector.tensor_mul(prod[:], h_cur[:], fcw[:])
            nc.vector.reduce_sum(dot[:], prod[:], axis=mybir.AxisListType.X)
            # realign the three 32-partition blocks onto partitions 0-31
            nc.sync.dma_start(al[:, 0:1], dot[0:32])
            nc.sync.dma_start(al[:, 1:2], dot[32:64])
            nc.sync.dma_start(al[:, 2:3], dot[64:96])
            nc.vector.tensor_copy(al[:, 3:4], fcb[:])
            nc.vector.reduce_sum(res[:], al[:], axis=mybir.AxisListType.X)
            nc.sync.dma_start(d_out.ap(), res[:])

    nc.compile()
    return nc


def _prep_inputs(t_steps, spatial, hour_idx, week_idx, time_emb, week_emb,
                 W_sp, U_sp, b_sp, W_h, U_h, b_h, W_w, U_w, b_w, fc_W, fc_b):
    perm = _gate_perm()
    f32 = np.float32

    def rw(m):  # reorder gate columns
        return np.ascontiguousarray(np.asarray(m, f32)[..., perm])

    u_sp = rw(U_sp)
    u_h = rw(U_h)
    u_w = rw(U_w)
    waug = rw(np.vstack([np.asarray(W_sp, f32), np.asarray(b_sp, f32)[None, :]]))
    txzh = rw(np.asarray(time_emb, f32) @ np.asarray(W_h, f32)
              + np.asarray(b_h, f32)[None, :])
    txzw = rw(np.asarray(week_emb, f32) @ np.asarray(W_w, f32)
              + np.asarray(b_w, f32)[None, :])

    fcw_t = np.asarray(fc_W, f32).reshape(3, H)  # chain c -> fc_W[c*H:(c+1)*H]
    fcw = np.repeat(fcw_t[:, None, :], BC, axis=1).reshape(96, H)
    fcw = np.ascontiguousarray(fcw)
    fcb = np.full((BC, 1), np.asarray(fc_b, f32).reshape(-1)[0], f32)

    spatial = np.asarray(spatial, f32)[:, :t_steps]
    hour_idx = np.asarray(hour_idx)[:, :t_steps]
    week_idx = np.asarray(week_idx)[:, :t_steps]

    eye24 = np.eye(24, dtype=f32)
    eye7 = np.eye(7, dtype=f32)

    in_maps = []
    for c in range(NCORES):
        bs = slice(c * BC, (c + 1) * BC)
        # xaug [T, 3, BC]: rows 0,1 spatial dims, row 2 ones
        xa = np.empty((t_steps, 3, BC), f32)
        xa[:, 0:2, :] = spatial[bs].transpose(1, 2, 0)
        xa[:, 2, :] = 1.0
        # one-hots [T, K, BC]
        ohh = np.ascontiguousarray(eye24[hour_idx[bs]].transpose(1, 2, 0))
        ohw = np.ascontiguousarray(eye7[week_idx[bs]].transpose(1, 2, 0))
        in_maps.append({
            "u_sp": u_sp, "u_h": u_h, "u_w": u_w, "waug": waug,
            "txzh": txzh, "txzw": txzw,
            "xaug": np.ascontiguousarray(xa), "ohh": ohh, "ohw": ohw,
            "fcw": fcw, "fcb": fcb,
        })
    return in_maps


def _run(t_steps, trace, inputs):
    from concourse import bass_utils

    key = t_steps
    if key not in _CACHE:
        _CACHE[key] = _build_program(t_steps)
    nc = _CACHE[key]

    in_maps = _prep_inputs(t_steps, **inputs)
    res = bass_utils.run_bass_kernel_spmd(
        nc, in_maps, core_ids=list(range(NCORES)), trace=trace,
    )
    out = np.concatenate(
        [res.results[c]["out"].reshape(BC) for c in range(NCORES)]
    ).astype(np.float32)
    return out, res


def kernel(**inputs) -> np.ndarray:
    out, _ = _run(T, False, inputs)
    return out
